# revision 22
# baseline (speedup 1.0000x reference)
"""Bass/Trainium2 kernel for nn_BiCRFModel: 2-layer BiLSTM + dense + CRF NLL.

Strategy (8-core data parallelism, 32 sequences/core), v2:
  - fw and bw LSTM runs are two INDEPENDENT instruction streams per layer
    (batch 32 in partitions each); their serial dependency chains interleave
    on the engines, hiding each other's latency.
  - All matmul operands bf16 (weights, hidden states, xw streams).
  - Gate-input projections (x @ Wx) as pre-GEMMs into DRAM xw tensors,
    emitted interleaved with the step loops so the Tile scheduler overlaps
    them into PE idle time.  Layer-0 bias rides a ones-row in the padded
    embT; layer-1 bias is a K=1 ones matmul.
  - The per-step "z = Wh@h + xw" add is folded into the PE accumulation via
    an identity-stationary matmul on the DMA-loaded xw tile.
  - No per-step masking: the pre-GEMM epilogue adds -1e9 to the i/f gate
    columns of xw rows with t >= seq_len, which forces c = h = 0 exactly at
    invalid steps (reproduces tf.reverse_sequence + output masking).
  - Layer-0 hT history is kept fully in SBUF; layer-1 pre-GEMM loads its
    stationary operands straight from it (no x1 DRAM round trip).
  - CRF forward recurrence in scaled-exp domain: per step one 9x9 matmul
    worth of work done as [32,81] DVE ops (5 ops, single engine), with
    per-sequence rescaling every RESCALE steps; log-scales accumulated.
Output: per-core sum of NLL over its 32 sequences; host sums and /256.
"""

import contextlib

import numpy as np

B, T, E, H, K = 256, 256, 300, 256, 9
N_CORES = 8

_CACHE = {}

NEG_BIG = -1e9
RESCALE = 16


# ---------------------------------------------------------------- wait split
def _split_excess_waits(nc, max_waits=1):
    """This walrus build allows only 1 sync wait per instruction.  Hoist
    excess waits onto InstEventSemaphore carriers inserted just before the
    instruction (same engine -> same program order -> identical blocking)."""
    import bass_rust
    import concourse.mybir as mybir

    n_split = 0
    for fn in nc.m.functions:
        for bb in fn.blocks:
            insts = list(bb.instructions)
            out = []
            changed = False
            for ins in insts:
                si = getattr(ins, "sync_info", None)
                waits = list(si.on_wait) if si is not None and si.on_wait else []
                if len(waits) > max_waits:
                    keep = waits[:max_waits]
                    rest = waits[max_waits:]
                    for ci in range(0, len(rest), max_waits):
                        nop = mybir.InstEventSemaphore(
                            name=f"{ins.name}-waitsplit-{ci}", ins=[], outs=[]
                        )
                        nop.engine = ins.engine
                        nop.bass_nofuse = True
                        nop.sync_info = bass_rust.SyncInfo(
                            on_wait=list(rest[ci : ci + max_waits]), on_update=[]
                        )
                        out.append(nop)
                    si.on_wait = keep
                    n_split += 1
                    changed = True
                out.append(ins)
            if changed:
                bb.instructions[:] = out
    return n_split


# ---------------------------------------------------------------- builder
def build_nc(cfg, split=True):
    import concourse.bass as bass
    import concourse.mybir as mybir
    from concourse import tile

    f32 = mybir.dt.float32
    bf16 = mybir.dt.bfloat16
    AF = mybir.ActivationFunctionType
    OP = mybir.AluOpType
    AX = mybir.AxisListType

    Tn = cfg["T"]
    BL = cfg["BL"]
    En = cfg["E"]
    Hn = cfg["H"]
    Kn = cfg["K"]
    EP = -(-En // 128) * 128            # padded input feat (includes ones row)
    G4 = 4 * Hn                         # gate width
    HP = 2 * Hn                         # concat feat
    NKE = EP // 128
    NKH = Hn // 128
    NKX = HP // 128
    ROWS = Tn * BL
    NCH = ROWS // 128                   # 128-row chunks
    TPC = 128 // BL                     # timesteps per chunk (4)
    K2 = Kn * Kn
    HT = NKH * BL                       # hT slot width (64)

    nc = bass.Bass("TRN2", num_devices=cfg["n_cores"])

    embT_d = nc.dram_tensor("embT", [EP, ROWS], bf16, kind="ExternalInput")
    mneg_d = nc.dram_tensor("mneg", [128, NCH], f32, kind="ExternalInput")
    id32_d = nc.dram_tensor("id32", [BL, BL], bf16, kind="ExternalInput")
    on1_d = nc.dram_tensor("ones1", [1, 128], bf16, kind="ExternalInput")
    dw_d = nc.dram_tensor("dwc", [128, NKX * Kn], bf16, kind="ExternalInput")
    db_d = nc.dram_tensor("db", [1, Kn], bf16, kind="ExternalInput")
    e81_d = nc.dram_tensor("e81", [BL, K2], f32, kind="ExternalInput")
    mcrf_d = nc.dram_tensor("mcrf", [BL, Tn], f32, kind="ExternalInput")
    oh_d = nc.dram_tensor("oh", [ROWS, Kn], f32, kind="ExternalInput")
    c81_d = nc.dram_tensor("c81t", [K2, BL], f32, kind="ExternalInput")
    sel_d = nc.dram_tensor("sel", [128, BL], f32, kind="ExternalInput")
    tf_d = nc.dram_tensor("transflat", [K2, 1], f32, kind="ExternalInput")
    wx_d, wh_d, b1_d = {}, {}, {}
    for d in ("f", "b"):
        wx_d[(0, d)] = nc.dram_tensor(f"wx0{d}", [128, NKE * G4], bf16, kind="ExternalInput")
        wx_d[(1, d)] = nc.dram_tensor(f"wx1{d}", [128, NKX * G4], bf16, kind="ExternalInput")
        b1_d[d] = nc.dram_tensor(f"b1{d}", [1, G4], bf16, kind="ExternalInput")
        for l in (0, 1):
            wh_d[(l, d)] = nc.dram_tensor(f"wh{l}{d}", [128, NKH * G4], bf16, kind="ExternalInput")
    out_d = nc.dram_tensor("out", [1, 1], f32, kind="ExternalOutput")

    with tile.TileContext(nc) as tc, contextlib.ExitStack() as ctx:
        cp = ctx.enter_context(tc.tile_pool(name="const", bufs=1))
        gp = ctx.enter_context(tc.tile_pool(name="work", bufs=2))
        sp = ctx.enter_context(tc.tile_pool(name="step", bufs=2))
        pzf = ctx.enter_context(tc.tile_pool(name="pzf", bufs=1, space="PSUM"))
        pzb = ctx.enter_context(tc.tile_pool(name="pzb", bufs=1, space="PSUM"))
        pg = ctx.enter_context(tc.tile_pool(name="pgemm", bufs=2, space="PSUM"))
        ps = ctx.enter_context(tc.tile_pool(name="psmall", bufs=2, space="PSUM"))
        dp = ctx.enter_context(tc.tile_pool(name="dram", bufs=1, space="DRAM"))

        def cload(name, dram, shape, dt=f32):
            t = cp.tile(shape, dt, name=name, tag=name)
            nc.sync.dma_start(t[:], dram[:, :])
            return t

        mneg = cload("mnegs", mneg_d, [128, NCH])
        id32 = cload("id32s", id32_d, [BL, BL], bf16)
        ones1 = cload("on1s", on1_d, [1, 128], bf16)
        e81 = cload("e81s", e81_d, [BL, K2])
        mcrf = cload("mcrfs", mcrf_d, [BL, Tn])
        c81s = cload("c81s", c81_d, [K2, BL])
        sels = cload("sels", sel_d, [128, BL])
        tfs = cload("tfs", tf_d, [K2, 1])
        dws = cload("dws", dw_d, [128, NKX * Kn], bf16)
        dbs = cload("dbs", db_d, [1, Kn], bf16)
        wxs, whs, b1s = {}, {}, {}
        for d in ("f", "b"):
            wxs[(0, d)] = cload(f"wx0{d}s", wx_d[(0, d)], [128, NKE * G4], bf16)
            wxs[(1, d)] = cload(f"wx1{d}s", wx_d[(1, d)], [128, NKX * G4], bf16)
            b1s[d] = cload(f"b1{d}s", b1_d[d], [1, G4], bf16)
            for l in (0, 1):
                whs[(l, d)] = cload(f"wh{l}{d}s", wh_d[(l, d)], [128, NKH * G4], bf16)
        onesb = cp.tile([BL, 1], f32, name="onesb", tag="onesb")
        nc.vector.memset(onesb[:], 1.0)

        # xw DRAM tensors, rows (t, b)
        xw = {}
        for l in (0, 1):
            for d in ("f", "b"):
                xw[(l, d)] = dp.tile([ROWS, G4], bf16, name=f"xw{l}{d}", tag=f"xw{l}{d}")
        x2t = dp.tile([HP, ROWS], bf16, name="x2t", tag="x2t")
        lgd = dp.tile([ROWS, Kn], f32, name="lgd", tag="lgd")

        # hT histories, TIME-indexed slots:
        #   fw: slot t+1 = h after time t, slot 0 = init zeros
        #   bw: slot t   = h after time t (processed descending), slot T = init
        # layer 0 keeps the full history (layer-1 pre-GEMM reads it);
        # layer 1 keeps a 4-slot ring (indices mod 4; T % 4 == 0).
        hist0, hist1 = {}, {}
        for d in ("f", "b"):
            h = cp.tile([128, (Tn + 1) * HT], bf16, name=f"hist0{d}", tag=f"hist0{d}")
            hist0[d] = h[:].rearrange("p (k t b) -> p k t b", t=Tn + 1, k=NKH)
            nc.vector.memset(hist0[d][:, :, 0 if d == "f" else Tn], 0.0)
            h1 = cp.tile([128, 8 * HT], bf16, name=f"hist1{d}", tag=f"hist1{d}")
            hist1[d] = h1[:].rearrange("p (k t b) -> p k t b", t=8, k=NKH)
            nc.vector.memset(hist1[d][:, :, 7 if d == "f" else Tn % 8], 0.0)

        # ---------------- pre-GEMM for one (layer, dir, chunk): xw rows 128c..
        def pre_gemm(l, d, c, nk):
            if l == 0:
                xt = gp.tile([128, nk * 128], bf16, name=f"pgx{l}{d}{c}", tag="pgx", bufs=3)
                nc.gpsimd.dma_start(
                    xt[:].rearrange("p (k c) -> p k c", k=nk),
                    embT_d[:, 128 * c : 128 * (c + 1)].rearrange("(k p) c -> p k c", p=128),
                )
                lhs = [xt[:, 128 * k : 128 * (k + 1)] for k in range(nk)]
            else:
                # chunk c covers times 4c..4c+3: fw slots 4c+1..4c+4, bw slots 4c..4c+3
                lhs = []
                for kc in range(NKX):
                    fw = kc < NKH
                    src = hist0["f"] if fw else hist0["b"]
                    s0 = TPC * c + (1 if fw else 0)
                    lhs.append(
                        src[:, kc % NKH, s0 : s0 + TPC, :].rearrange("p t b -> p (t b)")
                    )
            zs = gp.tile([128, G4], bf16, name=f"pgs{l}{d}{c}", tag="pgzs", bufs=3)
            for n in range(2):
                n0 = 512 * n
                zp = pg.tile([128, 512], f32, name=f"pg{l}{d}{c}n{n}", tag="pgz")
                first = True
                if l == 1:
                    nc.tensor.matmul(
                        zp[:], ones1[:], b1s[d][:, n0 : n0 + 512],
                        start=True, stop=False,
                    )
                    first = False
                for k in range(nk):
                    nc.tensor.matmul(
                        zp[:],
                        lhs[k],
                        wxs[(l, d)][:, k * G4 + n0 : k * G4 + n0 + 512],
                        start=first, stop=(k == nk - 1),
                    )
                    first = False
                if n == 0:
                    # copy + doctor i/f gates at invalid rows (+ -1e9 where t >= len)
                    nc.vector.tensor_scalar_add(
                        zs[:, 0:512], zp[:], mneg[:, c : c + 1]
                    )
                else:
                    nc.scalar.copy(zs[:, 512:], zp[:])
            nc.sync.dma_start(xw[(l, d)][128 * c : 128 * (c + 1), :], zs[:])

        # ---------------- one LSTM stream step (layer l, dir d, step s)
        # state per stream kept in python dict st: gc tile (g|c), hist view
        def make_stream(l, d):
            # [g | c] tile: cols 0:H = tanh(z_j), cols H:2H = cell state
            gc = cp.tile([BL, 2 * Hn], bf16, name=f"gc{l}{d}", tag=f"gc{l}{d}")
            nc.vector.memset(gc[:, Hn : 2 * Hn], 0.0)
            return {
                "gc": gc,
                "hist": hist0[d] if l == 0 else hist1[d],
                "nslot": Tn + 1 if l == 0 else 8,
                "pz": pzf if d == "f" else pzb,
            }

        def step(l, d, s, st):
            t = s if d == "f" else Tn - 1 - s
            hist, nslot = st["hist"], st["nslot"]
            if nslot == 8:
                # layer 1: slot t%8 = h after time t; fw reads (t-1)%8 (init
                # slot 7), bw reads (t+1)%8 (init slot Tn%8 == 0)
                sl_prev = (t - 1) % 8 if d == "f" else (t + 1) % 8
                sl_new = t % 8
            else:
                # layer 0: fw slot t+1 = h after t (init 0); bw slot t = h
                # after t (init Tn)
                sl_prev = t if d == "f" else t + 1
                sl_new = t + 1 if d == "f" else t

            if s % TPC == 0:
                # one chunk covers the next 4 timesteps of this stream,
                # timesteps along the free dim (base partition must be 0)
                q = t // TPC
                xwc = sp.tile([BL, TPC * G4], bf16, name=f"xwc{l}{d}{s}", tag=f"xwc{d}", bufs=2)
                nc.gpsimd.dma_start(
                    xwc[:].rearrange("b (t g) -> b t g", t=TPC),
                    xw[(l, d)][128 * q : 128 * (q + 1), :].rearrange(
                        "(t b) g -> b t g", b=BL
                    ),
                )
                st["xwc"] = xwc
            xwt = st["xwc"][:, (t % TPC) * G4 : (t % TPC + 1) * G4]

            zp = st["pz"].tile([BL, G4], f32, name=f"z{l}{d}{s}", tag=f"zp{d}")
            for n in (1, 0):  # (o,j) slice first so tanh_j overlaps (i,f) mms
                n0 = 512 * n
                for k in range(NKH):
                    nc.tensor.matmul(
                        zp[:, n0 : n0 + 512],
                        hist[:, k, sl_prev, :],
                        whs[(l, d)][:, k * G4 + n0 : k * G4 + n0 + 512],
                        start=(k == 0), stop=False,
                    )
                nc.tensor.matmul(
                    zp[:, n0 : n0 + 512], id32[:], xwt[:, n0 : n0 + 512],
                    start=False, stop=True,
                )

            # o-gate cols prescaled 0.5 on host: one tanh covers (t_o, g);
            # hhat = (t_o+1)*tanh(c) = 2h, 0.5 folded into h's consumers.
            gc = st["gc"]
            toj = sp.tile([BL, 2 * Hn], bf16, name=f"toj{l}{d}{s}", tag=f"toj{d}")
            nc.scalar.activation(toj[:], zp[:, 2 * Hn : 4 * Hn], AF.Tanh)
            sio = sp.tile([BL, 2 * Hn], bf16, name=f"sio{l}{d}{s}", tag=f"sio{d}")
            nc.scalar.activation(sio[:], zp[:, 0 : 2 * Hn], AF.Sigmoid)
            nc.vector.tensor_copy(gc[:, 0:Hn], toj[:, Hn : 2 * Hn])
            prods = sp.tile([BL, 2 * Hn], bf16, name=f"pr{l}{d}{s}", tag=f"pr{d}")
            nc.vector.tensor_tensor(prods[:], sio[:], gc[:], op=OP.mult)
            nc.vector.tensor_tensor(
                gc[:, Hn : 2 * Hn], prods[:, 0:Hn], prods[:, Hn : 2 * Hn], op=OP.add
            )
            th = sp.tile([BL, Hn], bf16, name=f"th{l}{d}{s}", tag=f"th{d}")
            nc.scalar.activation(th[:], gc[:, Hn : 2 * Hn], AF.Tanh)
            h = sp.tile([BL, Hn], bf16, name=f"h{l}{d}{s}", tag=f"h{d}")
            nc.vector.scalar_tensor_tensor(
                h[:], toj[:, 0:Hn], 1.0, th[:], OP.add, OP.mult
            )

            tp = ps.tile([128, HT], bf16, name=f"tp{l}{d}{s}", tag="sm")
            for k in range(NKH):
                nc.tensor.matmul(
                    tp[:, BL * k : BL * (k + 1)],
                    h[:, 128 * k : 128 * (k + 1)],
                    id32[:],
                    is_transpose=True,
                )
            nc.vector.tensor_copy(
                hist[:, :, sl_new, :], tp[:].rearrange("p (k b) -> p k b", k=NKH)
            )
            if l == 1 and s % TPC == TPC - 1:
                # store the finished 4-slot group into x2t (fw rows 0:Hn,
                # bw rows Hn:2Hn); group start tg has tg % 4 == 0
                tg = t - 3 if d == "f" else t
                ro = 0 if d == "f" else Hn
                g0 = tg % 8
                nc.sync.dma_start(
                    x2t[ro : ro + Hn, BL * tg : BL * (tg + 4)].rearrange(
                        "(k p) (t b) -> p k t b", p=128, b=BL
                    ),
                    hist[:, :, g0 : g0 + 4, :],
                )

        # ---------------- logits chunk + unary accum
        usum = cp.tile([128, NCH], f32, name="usum", tag="usum")

        def logits_chunk(c):
            xt = gp.tile([128, NKX * 128], bf16, name=f"lgx{c}", tag="lgx", bufs=3)
            nc.gpsimd.dma_start(
                xt[:].rearrange("p (k c) -> p k c", k=NKX),
                x2t[:, 128 * c : 128 * (c + 1)].rearrange("(k p) c -> p k c", p=128),
            )
            lp = ps.tile([128, Kn], f32, name=f"lp{c}", tag="sm")
            nc.tensor.matmul(lp[:], ones1[:], dbs[:], start=True, stop=False)
            for k in range(NKX):
                nc.tensor.matmul(
                    lp[:], xt[:, 128 * k : 128 * (k + 1)], dws[:, Kn * k : Kn * (k + 1)],
                    start=False, stop=(k == NKX - 1),
                )
            lgc = gp.tile([128, Kn], f32, name=f"lg{c}", tag="lgc", bufs=3)
            nc.vector.tensor_copy(lgc[:], lp[:])
            nc.sync.dma_start(lgd[128 * c : 128 * (c + 1), :], lgc[:])
            oht = gp.tile([128, Kn], f32, name=f"oht{c}", tag="oht", bufs=3)
            nc.gpsimd.dma_start(oht[:], oh_d[128 * c : 128 * (c + 1), :])
            scr = gp.tile([128, Kn], f32, name=f"ohscr{c}", tag="ohscr", bufs=3)
            nc.vector.scalar_tensor_tensor(
                scr[:], lgc[:], 1.0, oht[:], OP.mult, OP.mult,
                accum_out=usum[:, c : c + 1],
            )

        # ---------------- CRF (scaled-exp domain)
        def crf():
            crfp = ctx.enter_context(tc.tile_pool(name="crf", bufs=2))
            lgall = cp.tile([BL, Tn * Kn], f32, name="lgall", tag="lgall")
            nc.sync.dma_start(
                lgall[:].rearrange("b (t k) -> b t k", k=Kn),
                lgd[:, :].rearrange("(t b) k -> b t k", b=BL),
            )
            expx = cp.tile([BL, Tn * Kn], f32, name="expx", tag="expx")
            nc.scalar.activation(expx[:], lgall[:], AF.Exp)
            ls = cp.tile([BL, 1], f32, name="ls", tag="ls")
            nc.vector.memset(ls[:], 0.0)
            a = crfp.tile([BL, Kn], f32, name="a0", tag="a")
            nc.vector.tensor_copy(a[:], expx[:, 0:Kn])
            for t in range(1, Tn):
                p81 = crfp.tile([BL, K2], f32, name=f"p81_{t}", tag="p81")
                nc.vector.tensor_tensor(
                    p81[:].rearrange("p (j i) -> p j i", i=Kn),
                    a[:].unsqueeze(1).broadcast_to([BL, Kn, Kn]),
                    e81[:].rearrange("p (j i) -> p j i", i=Kn),
                    op=OP.mult,
                )
                s9 = crfp.tile([BL, Kn], f32, name=f"s9_{t}", tag="s9")
                nc.vector.reduce_sum(
                    s9[:], p81[:].rearrange("p (j i) -> p j i", i=Kn), axis=AX.X
                )
                u = crfp.tile([BL, Kn], f32, name=f"u{t}", tag="u")
                nc.vector.tensor_tensor(u[:], s9[:], expx[:, Kn * t : Kn * (t + 1)], op=OP.mult)
                dd = crfp.tile([BL, Kn], f32, name=f"dd{t}", tag="dd")
                nc.vector.tensor_tensor(dd[:], u[:], a[:], op=OP.subtract)
                anew = crfp.tile([BL, Kn], f32, name=f"a{t}", tag="a")
                nc.vector.scalar_tensor_tensor(
                    anew[:], dd[:], mcrf[:, t : t + 1], a[:], OP.mult, OP.add
                )
                a = anew
                if t % RESCALE == 0:
                    mx = crfp.tile([BL, 1], f32, name=f"mx{t}", tag="mx")
                    nc.vector.reduce_max(mx[:], a[:], axis=AX.X)
                    r = crfp.tile([BL, 1], f32, name=f"r{t}", tag="r")
                    nc.vector.reciprocal(r[:], mx[:])
                    ar = crfp.tile([BL, Kn], f32, name=f"ar{t}", tag="a")
                    nc.vector.tensor_scalar_mul(ar[:], a[:], r[:, 0:1])
                    a = ar
                    lmx = crfp.tile([BL, 1], f32, name=f"lmx{t}", tag="lmx")
                    nc.scalar.activation(lmx[:], mx[:], AF.Ln)
                    nc.vector.tensor_tensor(ls[:], ls[:], lmx[:], op=OP.add)

            # logZ = ls + ln(sum a)
            sa = crfp.tile([BL, 1], f32, name="sa", tag="sa")
            nc.vector.reduce_sum(sa[:], a[:], axis=AX.X)
            lsa = crfp.tile([BL, 1], f32, name="lsa", tag="lsa")
            nc.scalar.activation(lsa[:], sa[:], AF.Ln)
            lgz = crfp.tile([BL, 1], f32, name="lgz", tag="lgz")
            nc.vector.tensor_tensor(lgz[:], lsa[:], ls[:], op=OP.add)

            # gold path scores
            up = ps.tile([BL, NCH], f32, name="up", tag="sm")
            nc.tensor.matmul(up[:], sels[:], usum[:], start=True, stop=True)
            unary = crfp.tile([BL, 1], f32, name="unary", tag="unary")
            nc.vector.reduce_sum(unary[:], up[:], axis=AX.X)
            bp = ps.tile([BL, 1], f32, name="bp", tag="sm")
            nc.tensor.matmul(bp[:], c81s[:], tfs[:], start=True, stop=True)
            binry = crfp.tile([BL, 1], f32, name="binry", tag="binry")
            nc.scalar.copy(binry[:], bp[:])

            zb = crfp.tile([BL, 1], f32, name="zb", tag="zb")
            nc.vector.tensor_tensor(zb[:], lgz[:], unary[:], op=OP.subtract)
            nll = crfp.tile([BL, 1], f32, name="nll", tag="nll")
            nc.vector.tensor_tensor(nll[:], zb[:], binry[:], op=OP.subtract)
            pf = ps.tile([1, 1], f32, name="pf", tag="sm")
            nc.tensor.matmul(pf[:], nll[:], onesb[:], start=True, stop=True)
            osb = crfp.tile([1, 1], f32, name="osb", tag="osb")
            nc.scalar.copy(osb[:], pf[:])
            nc.sync.dma_start(out_d[:, :], osb[:])

        # ---------------- emission schedule
        PH = cfg.get("phase", 99)
        NSTEP = min(Tn, cfg.get("nsteps", Tn))

        def probe(src_ap):
            pt = cp.tile([1, 1], f32, name="probe", tag="probe")
            nc.sync.dma_start(pt[:], src_ap)
            nc.sync.dma_start(out_d[:, :], pt[:])

        PRE = 6
        # prologue: first layer-0 pre-GEMM chunks
        for ci in range(min(PRE, NCH)):
            pre_gemm(0, "f", ci, NKE)
            pre_gemm(0, "b", NCH - 1 - ci, NKE)

        def l0_hook(s):
            if s % TPC == TPC - 1:
                ci = s // TPC + PRE
                if ci < NCH:
                    pre_gemm(0, "f", ci, NKE)
                    pre_gemm(0, "b", NCH - 1 - ci, NKE)
            if PH >= 3 and s >= 131 and (s - 131) % TPC == 0:
                j = (s - 131) // TPC
                for q in (NCH // 2 - 1 - j, NCH // 2 + j):
                    if 0 <= q < NCH:
                        pre_gemm(1, "f", q, NKX)
                        pre_gemm(1, "b", q, NKX)

        def l1_hook(s):
            if PH >= 4 and s >= 131 and (s - 131) % TPC == 0:
                j = (s - 131) // TPC
                for q in (NCH // 2 - 1 - j, NCH // 2 + j):
                    if 0 <= q < NCH:
                        logits_chunk(q)

        def run_layer(l, hook):
            sts = {d: make_stream(l, d) for d in ("f", "b")}
            for s in range(NSTEP):
                for d in ("f", "b"):
                    step(l, d, s, sts[d])
                hook(s)

        run_layer(0, l0_hook)
        if PH == 1:
            pr = cp.tile([1, 1], f32, name="prcv", tag="prcv")
            nc.vector.tensor_copy(pr[:], hist0["f"][0:1, 0, Tn, 0:1])
            nc.sync.dma_start(out_d[:, :], pr[:])
        if PH >= 3:
            run_layer(1, l1_hook)
        if PH == 4:
            probe(lgd[0:1, 0:1])
        if PH >= 5:
            crf()

    if split:
        _split_excess_waits(nc)
    return nc


# ---------------------------------------------------------------- host prep
def _prep_core(emb_c, lens_c, tgt_c, weights, cfg):
    Tn, BL, En, Hn, Kn = cfg["T"], cfg["BL"], cfg["E"], cfg["H"], cfg["K"]
    EP = -(-En // 128) * 128
    G4 = 4 * Hn
    HP = 2 * Hn
    NKE = EP // 128
    NKX = HP // 128
    ROWS = Tn * BL
    NCH = ROWS // 128
    K2 = Kn * Kn

    perm = np.concatenate(
        [np.arange(0, Hn), np.arange(2 * Hn, 3 * Hn),
         np.arange(3 * Hn, 4 * Hn), np.arange(Hn, 2 * Hn)]
    )

    def prep_wb(w, b):
        wp = np.ascontiguousarray(w[:, perm], np.float32)
        bp = b[perm].astype(np.float32).copy()
        bp[Hn : 2 * Hn] += 1.0
        return wp, bp

    def chunk_k(w, kpad):
        out = np.zeros((kpad, w.shape[1]), np.float32)
        out[: w.shape[0]] = w
        nk = kpad // 128
        return np.ascontiguousarray(
            out.reshape(nk, 128, w.shape[1]).transpose(1, 0, 2).reshape(128, -1)
        )

    d = {}
    et = emb_c.transpose(2, 1, 0).reshape(En, ROWS)
    embT = np.zeros((EP, ROWS), np.float32)
    embT[:En] = et
    embT[En] = 1.0  # ones row for layer-0 bias
    d["embT"] = embT.astype(np.float32)

    tt = np.arange(Tn)
    valid = tt[:, None] < lens_c[None, :]          # [T, BL]
    # mneg rows (t, b): chunk c covers t = 4c..4c+3; partition p = (t%4)*32+b
    mneg = np.where(valid, 0.0, NEG_BIG).astype(np.float32)  # [T, BL]
    d["mneg"] = np.ascontiguousarray(
        mneg.reshape(NCH, 128 // BL, BL).transpose(1, 2, 0).reshape(128, NCH)
    )
    d["id32"] = np.eye(BL, dtype=np.float32)
    d["ones1"] = np.ones((1, 128), np.float32)
    d["dwc"] = chunk_k(weights["dense_w"].astype(np.float32), HP)
    d["db"] = weights["dense_b"].reshape(1, Kn).astype(np.float32)

    trans = weights["trans"].astype(np.float32)
    # p81[p, j, i] = a_i * e81[(j, i)], reduce over i (AX.X = innermost):
    # s9[p, j] = sum_i a_i exp(trans[i, j])  ->  e81[(j, i)] = exp(trans)[i, j]
    d["e81"] = np.tile(np.exp(trans).T.reshape(1, K2), (BL, 1)).astype(np.float32)
    d["mcrf"] = valid.T.astype(np.float32).copy()  # [BL, T]

    ohm = np.zeros((ROWS, Kn), np.float32)
    r = tt[:, None] * BL + np.arange(BL)[None, :]
    ohm[r.ravel(), tgt_c.T.ravel()] = valid.astype(np.float32).ravel()
    d["oh"] = ohm
    c81 = np.zeros((K2, BL), np.float32)
    for b in range(BL):
        L = int(lens_c[b])
        for t in range(L - 1):
            c81[tgt_c[b, t] * Kn + tgt_c[b, t + 1], b] += 1.0
    d["c81t"] = c81
    d["sel"] = (np.arange(128)[:, None] % BL == np.arange(BL)[None, :]).astype(np.float32)
    d["transflat"] = trans.reshape(K2, 1)

    for l, (wfk, bfk, wbk, bbk, kin) in enumerate(
        (("w_fw0", "b_fw0", "w_bw0", "b_bw0", EP), ("w_fw1", "b_fw1", "w_bw1", "b_bw1", HP))
    ):
        for dd, (wk, bk) in (("f", (wfk, bfk)), ("b", (wbk, bbk))):
            w, b = prep_wb(weights[wk], weights[bk])
            nin = w.shape[0] - Hn
            wx_part = w[:nin].copy()
            wh_part = w[nin:]
            if l == 0:
                wx_pad = np.zeros((kin, G4), np.float32)
                wx_pad[:nin] = wx_part
                wx_pad[En] = b  # bias via ones row
                d[f"wx{l}{dd}"] = chunk_k(wx_pad, kin)
            else:
                d[f"wx{l}{dd}"] = chunk_k(wx_part, kin)
                d[f"b1{dd}"] = b.reshape(1, G4)
            d[f"wh{l}{dd}"] = chunk_k(wh_part, Hn)
    return d


_BF16_INPUTS = {
    "embT", "id32", "ones1", "dwc", "db",
    "wx0f", "wx0b", "wx1f", "wx1b", "b1f", "b1b",
    "wh0f", "wh0b", "wh1f", "wh1b",
}


def _get_runner(cfg):
    key = ("runner", cfg["T"], cfg["BL"], cfg["n_cores"], cfg.get("phase", 99))
    if key in _CACHE:
        return _CACHE[key]
    nc = build_nc(cfg)
    from concourse import bass2jax

    n_cores = cfg["n_cores"]

    import jax
    import numpy as _np
    from jax.sharding import Mesh, PartitionSpec
    from jax.experimental.shard_map import shard_map

    bass2jax.install_neuronx_cc_hook()
    partition_name = nc.partition_id_tensor.name if nc.partition_id_tensor else None
    import concourse.mybir as mybir

    in_names, out_names, out_avals = [], [], []
    in_dtypes = {}
    for alloc in nc.m.functions[0].allocations:
        if not isinstance(alloc, mybir.MemoryLocationSet):
            continue
        name = alloc.memorylocations[0].name
        if alloc.kind == "ExternalInput":
            if name != partition_name:
                in_names.append(name)
                in_dtypes[name] = mybir.dt.np(alloc.dtype)
        elif alloc.kind == "ExternalOutput":
            out_names.append(name)
            out_avals.append(
                jax.core.ShapedArray(tuple(alloc.tensor_shape), mybir.dt.np(alloc.dtype))
            )
    n_params = len(in_names)
    all_names = in_names + out_names
    if partition_name is not None:
        all_names = all_names + [partition_name]
    donate = tuple(range(n_params, n_params + len(out_names)))

    def _body(*args):
        operands = list(args)
        if partition_name is not None:
            operands.append(bass2jax.partition_id_tensor())
        outs = bass2jax._bass_exec_p.bind(
            *operands,
            out_avals=tuple(out_avals),
            in_names=tuple(all_names),
            out_names=tuple(out_names),
            lowering_input_output_aliases=(),
            sim_require_finite=True,
            sim_require_nnan=True,
            nc=nc,
        )
        return tuple(outs)

    devices = jax.devices()[:n_cores]

    class Runner:
        pass

    r = Runner()
    r.in_names, r.out_names, r.out_avals, r.n_cores = in_names, out_names, out_avals, n_cores
    r.in_dtypes = in_dtypes
    if n_cores == 1:
        fn = jax.jit(_body, donate_argnums=donate, keep_unused=True)

        def pack(in_maps):
            return [np.asarray(in_maps[0][n], in_dtypes[n]) for n in in_names]

        def call(packed):
            zeros = [np.zeros(a.shape, a.dtype) for a in out_avals]
            outs = fn(*packed, *zeros)
            return [{n: np.asarray(outs[i]) for i, n in enumerate(out_names)}]
    else:
        from jax.sharding import NamedSharding

        mesh = Mesh(_np.asarray(devices), ("core",))
        fn = jax.jit(
            shard_map(
                _body,
                mesh=mesh,
                in_specs=(PartitionSpec("core"),) * (n_params + len(out_names)),
                out_specs=(PartitionSpec("core"),) * len(out_names),
                check_rep=False,
            ),
            donate_argnums=donate,
            keep_unused=True,
        )
        sh = NamedSharding(mesh, PartitionSpec("core"))

        def pack(in_maps):
            concat_in = [
                np.concatenate(
                    [np.asarray(m[n], in_dtypes[n]) for m in in_maps], axis=0
                )
                for n in in_names
            ]
            return [jax.device_put(a, sh) for a in concat_in]

        def call(packed):
            zeros = [
                np.zeros((n_cores * a.shape[0],) + tuple(a.shape[1:]), a.dtype)
                for a in out_avals
            ]
            outs = fn(*packed, *zeros)
            return [
                {
                    n: np.asarray(outs[i]).reshape((n_cores,) + tuple(out_avals[i].shape))[c]
                    for i, n in enumerate(out_names)
                }
                for c in range(n_cores)
            ]

    r.fn = fn
    r.pack = pack
    r.call = call

    def run(in_maps):
        return call(pack(in_maps))

    r.run = run
    _CACHE[key] = r
    return r


def make_in_maps(inputs, cfg):
    n_cores = cfg["n_cores"]
    BL = cfg["BL"]
    weights = {
        k: np.asarray(inputs[k], np.float32)
        for k in (
            "w_fw0", "b_fw0", "w_bw0", "b_bw0",
            "w_fw1", "b_fw1", "w_bw1", "b_bw1",
            "dense_w", "dense_b", "trans",
        )
    }
    emb = np.asarray(inputs["emb"], np.float32)
    lens = np.asarray(inputs["seq_lens"], np.int64)
    tgt = np.asarray(inputs["targets"], np.int64)
    in_maps = []
    for c in range(n_cores):
        sl = slice(c * BL, (c + 1) * BL)
        in_maps.append(_prep_core(emb[sl], lens[sl], tgt[sl], weights, cfg))
    return in_maps


def kernel(**inputs):
    cfg = dict(T=T, BL=B // N_CORES, E=E, H=H, K=K, n_cores=N_CORES)
    in_maps = make_in_maps(inputs, cfg)
    runner = _get_runner(cfg)
    res = runner.run(in_maps)
    total = sum(float(r["out"][0, 0]) for r in res)
    return np.asarray(np.float32(total / B))


# revision 43
# speedup vs baseline: 2.0762x; 2.0762x over previous
"""Bass/Trainium2 kernel for nn_BiCRFModel: 2-layer BiLSTM + dense + CRF NLL.

Strategy (8-core data parallelism, 32 sequences/core), v2:
  - fw and bw LSTM runs are two INDEPENDENT instruction streams per layer
    (batch 32 in partitions each); their serial dependency chains interleave
    on the engines, hiding each other's latency.
  - All matmul operands bf16 (weights, hidden states, xw streams).
  - Gate-input projections (x @ Wx) as pre-GEMMs into DRAM xw tensors,
    emitted interleaved with the step loops so the Tile scheduler overlaps
    them into PE idle time.  Layer-0 bias rides a ones-row in the padded
    embT; layer-1 bias is a K=1 ones matmul.
  - The per-step "z = Wh@h + xw" add is folded into the PE accumulation via
    an identity-stationary matmul on the DMA-loaded xw tile.
  - No per-step masking: the pre-GEMM epilogue adds -1e9 to the i/f gate
    columns of xw rows with t >= seq_len, which forces c = h = 0 exactly at
    invalid steps (reproduces tf.reverse_sequence + output masking).
  - Layer-0 hT history is kept fully in SBUF; layer-1 pre-GEMM loads its
    stationary operands straight from it (no x1 DRAM round trip).
  - CRF forward recurrence in scaled-exp domain: per step one 9x9 matmul
    worth of work done as [32,81] DVE ops (5 ops, single engine), with
    per-sequence rescaling every RESCALE steps; log-scales accumulated.
Output: per-core sum of NLL over its 32 sequences; host sums and /256.
"""

import contextlib

import numpy as np

B, T, E, H, K = 256, 256, 300, 256, 9
N_CORES = 8

_CACHE = {}

NEG_BIG = -1e9
RESCALE = 16


# ---------------------------------------------------------------- wait split
def _split_excess_waits(nc, max_waits=1):
    """This walrus build allows only 1 sync wait per instruction.  Hoist
    excess waits onto InstEventSemaphore carriers inserted just before the
    instruction (same engine -> same program order -> identical blocking)."""
    import bass_rust
    import concourse.mybir as mybir

    n_split = 0
    for fn in nc.m.functions:
        for bb in fn.blocks:
            insts = list(bb.instructions)
            out = []
            changed = False
            for ins in insts:
                si = getattr(ins, "sync_info", None)
                waits = list(si.on_wait) if si is not None and si.on_wait else []
                if len(waits) > max_waits:
                    keep = waits[:max_waits]
                    rest = waits[max_waits:]
                    for ci in range(0, len(rest), max_waits):
                        nop = mybir.InstEventSemaphore(
                            name=f"{ins.name}-waitsplit-{ci}", ins=[], outs=[]
                        )
                        nop.engine = ins.engine
                        nop.bass_nofuse = True
                        nop.sync_info = bass_rust.SyncInfo(
                            on_wait=list(rest[ci : ci + max_waits]), on_update=[]
                        )
                        out.append(nop)
                    si.on_wait = keep
                    n_split += 1
                    changed = True
                out.append(ins)
            if changed:
                bb.instructions[:] = out
    return n_split


# ---------------------------------------------------------------- builder
def build_nc(cfg, split=True):
    import concourse.bass as bass
    import concourse.mybir as mybir
    from concourse import tile

    f32 = mybir.dt.float32
    bf16 = mybir.dt.bfloat16
    AF = mybir.ActivationFunctionType
    OP = mybir.AluOpType
    AX = mybir.AxisListType

    Tn = cfg["T"]
    BL = cfg["BL"]
    En = cfg["E"]
    Hn = cfg["H"]
    Kn = cfg["K"]
    EP = -(-En // 128) * 128            # padded input feat (includes ones row)
    G4 = 4 * Hn                         # gate width
    HP = 2 * Hn                         # concat feat
    NKE = EP // 128
    NKH = Hn // 128
    NKX = HP // 128
    ROWS = Tn * BL
    NCH = ROWS // 128                   # 128-row chunks
    TPC = 128 // BL                     # timesteps per chunk (4)
    K2 = Kn * Kn
    HT = NKH * BL                       # hT slot width (64)

    nc = bass.Bass("TRN2", num_devices=cfg["n_cores"])

    embT_d = nc.dram_tensor("embT", [EP, ROWS], bf16, kind="ExternalInput")
    mneg_d = nc.dram_tensor("mneg", [128, NCH], f32, kind="ExternalInput")
    id32_d = nc.dram_tensor("id32", [BL, BL], bf16, kind="ExternalInput")
    on1_d = nc.dram_tensor("ones1", [1, 128], bf16, kind="ExternalInput")
    dw_d = nc.dram_tensor("dwc", [128, NKX * Kn], bf16, kind="ExternalInput")
    db_d = nc.dram_tensor("db", [1, Kn], bf16, kind="ExternalInput")
    e81_d = nc.dram_tensor("e81", [BL, K2], f32, kind="ExternalInput")
    m9_d = nc.dram_tensor("m9", [BL, Tn * Kn], mybir.dt.uint8, kind="ExternalInput")
    oh_d = nc.dram_tensor("oh", [ROWS, Kn], f32, kind="ExternalInput")
    c81_d = nc.dram_tensor("c81t", [K2, BL], f32, kind="ExternalInput")
    sel_d = nc.dram_tensor("sel", [128, BL], f32, kind="ExternalInput")
    tf_d = nc.dram_tensor("transflat", [K2, 1], f32, kind="ExternalInput")
    wx_d, wh_d, b1_d = {}, {}, {}
    for d in ("f", "b"):
        wx_d[(0, d)] = nc.dram_tensor(f"wx0{d}", [128, NKE * G4], bf16, kind="ExternalInput")
        wx_d[(1, d)] = nc.dram_tensor(f"wx1{d}", [128, NKX * G4], bf16, kind="ExternalInput")
        b1_d[d] = nc.dram_tensor(f"b1{d}", [128, G4], bf16, kind="ExternalInput")
        for l in (0, 1):
            wh_d[(l, d)] = nc.dram_tensor(f"wh{l}{d}", [128, NKH * G4], bf16, kind="ExternalInput")
    out_d = nc.dram_tensor("out", [1, 1], f32, kind="ExternalOutput")

    with tile.TileContext(nc) as tc, contextlib.ExitStack() as ctx:
        cp = ctx.enter_context(tc.tile_pool(name="const", bufs=1))
        gp = ctx.enter_context(tc.tile_pool(name="work", bufs=2))
        sp = ctx.enter_context(tc.tile_pool(name="step", bufs=2))
        pzf = ctx.enter_context(tc.tile_pool(name="pzf", bufs=1, space="PSUM"))
        pzb = ctx.enter_context(tc.tile_pool(name="pzb", bufs=1, space="PSUM"))
        pg = ctx.enter_context(tc.tile_pool(name="pgemm", bufs=2, space="PSUM"))
        ps = ctx.enter_context(tc.tile_pool(name="psmall", bufs=2, space="PSUM"))
        dp = ctx.enter_context(tc.tile_pool(name="dram", bufs=1, space="DRAM"))

        def cload(name, dram, shape, dt=f32):
            t = cp.tile(shape, dt, name=name, tag=name)
            nc.sync.dma_start(t[:], dram[:, :])
            return t

        # load order = scheduler priority: layer-0 weights gate all early
        # work, so they go first; layer-1/dense/CRF constants follow.
        wxs, whs, b1s = {}, {}, {}
        for d in ("f", "b"):
            wxs[(0, d)] = cload(f"wx0{d}s", wx_d[(0, d)], [128, NKE * G4], bf16)
        mneg = cload("mnegs", mneg_d, [128, NCH])
        id32 = cload("id32s", id32_d, [BL, BL], bf16)
        for d in ("f", "b"):
            whs[(0, d)] = cload(f"wh0{d}s", wh_d[(0, d)], [128, NKH * G4], bf16)
        ones1 = cload("on1s", on1_d, [1, 128], bf16)
        for d in ("f", "b"):
            wxs[(1, d)] = cload(f"wx1{d}s", wx_d[(1, d)], [128, NKX * G4], bf16)
            b1s[d] = cload(f"b1{d}s", b1_d[d], [128, G4], bf16)
            whs[(1, d)] = cload(f"wh1{d}s", wh_d[(1, d)], [128, NKH * G4], bf16)
        e81 = cload("e81s", e81_d, [BL, K2])
        m9 = cload("m9s", m9_d, [BL, Tn * Kn], mybir.dt.uint8)
        c81s = cload("c81s", c81_d, [K2, BL])
        sels = cload("sels", sel_d, [128, BL])
        tfs = cload("tfs", tf_d, [K2, 1])
        dws = cload("dws", dw_d, [128, NKX * Kn], bf16)
        dbs = cload("dbs", db_d, [1, Kn], bf16)
        onesb = cp.tile([BL, 1], f32, name="onesb", tag="onesb")
        nc.vector.memset(onesb[:], 1.0)

        lgd = dp.tile([ROWS, Kn], f32, name="lgd", tag="lgd")
        # xw DRAM tensors, rows (t, b)
        xw = {}
        for l in (0, 1):
            for d in ("f", "b"):
                xw[(l, d)] = dp.tile([ROWS, G4], bf16, name=f"xw{l}{d}", tag=f"xw{l}{d}")


        # hT histories, TIME-indexed slots (full history per layer):
        #   fw: slot t+1 = h after time t, slot 0 = init zeros
        #   bw: slot t   = h after time t (processed descending), slot T = init
        # layer-1 pre-GEMM reads hist0; the logits GEMM reads hist1 (both in
        # SBUF; hist0's pool is released before hist1 allocates).
        hp0cm = tc.tile_pool(name="hist0pool", bufs=1)
        hp0 = hp0cm.__enter__()
        hist0, hist1 = {}, {}
        for d in ("f", "b"):
            h = hp0.tile([128, (Tn + 1) * HT], bf16, name=f"hist0{d}", tag=f"hist0{d}")
            hist0[d] = h[:].rearrange("p (k t b) -> p k t b", t=Tn + 1, k=NKH)
            nc.vector.memset(hist0[d][:, :, 0 if d == "f" else Tn], 0.0)

        # ---------------- pre-GEMM for one (layer, dir, chunk): xw rows 128c..
        def pre_gemm(l, d, c, nk):
            if l == 0:
                xt = gp.tile([128, nk * 128], bf16, name=f"pgx{l}{d}{c}", tag="pgx", bufs=3)
                nc.gpsimd.dma_start(
                    xt[:].rearrange("p (k c) -> p k c", k=nk),
                    embT_d[:, 128 * c : 128 * (c + 1)].rearrange("(k p) c -> p k c", p=128),
                )
                lhs = [xt[:, 128 * k : 128 * (k + 1)] for k in range(nk)]
            else:
                # chunk c covers times 4c..4c+3: fw slots 4c+1..4c+4, bw slots 4c..4c+3
                lhs = []
                for kc in range(NKX):
                    fw = kc < NKH
                    src = hist0["f"] if fw else hist0["b"]
                    s0 = TPC * c + (1 if fw else 0)
                    lhs.append(
                        src[:, kc % NKH, s0 : s0 + TPC, :].rearrange("p t b -> p (t b)")
                    )
            zs = gp.tile([128, G4], bf16, name=f"pgs{l}{d}{c}", tag="pgzs", bufs=3)
            for n in range(2):
                n0 = 512 * n
                zp = pg.tile([128, 512], f32, name=f"pg{l}{d}{c}n{n}", tag="pgz")
                for k in range(nk):
                    nc.tensor.matmul(
                        zp[:],
                        lhs[k],
                        wxs[(l, d)][:, k * G4 + n0 : k * G4 + n0 + 512],
                        start=(k == 0), stop=(k == nk - 1),
                    )
                # epilogue: copy + layer-1 bias (pre-broadcast) + doctor the
                # i/f gate cols at invalid rows (+ -1e9 where t >= len)
                if l == 1:
                    if n == 0:
                        nc.vector.scalar_tensor_tensor(
                            zs[:, 0:512], zp[:], mneg[:, c : c + 1],
                            b1s[d][:, 0:512], OP.add, OP.add,
                        )
                    else:
                        nc.vector.tensor_tensor(
                            zs[:, 512:], zp[:], b1s[d][:, 512:], op=OP.add
                        )
                else:
                    if n == 0:
                        nc.vector.tensor_scalar_add(
                            zs[:, 0:512], zp[:], mneg[:, c : c + 1]
                        )
                    else:
                        nc.scalar.copy(zs[:, 512:], zp[:])
            nc.sync.dma_start(xw[(l, d)][128 * c : 128 * (c + 1), :], zs[:])

        # ---------------- one LSTM stream step (layer l, dir d, step s)
        # state per stream kept in python dict st: gc tile (g|c), hist view
        def make_stream(l, d):
            # [g | c] tile: cols 0:H = tanh(z_j), cols H:2H = cell state
            gc = cp.tile([BL, 2 * Hn], bf16, name=f"gc{l}{d}", tag=f"gc{l}{d}")
            nc.vector.memset(gc[:, Hn : 2 * Hn], 0.0)
            return {
                "gc": gc,
                "hist": hist0[d] if l == 0 else hist1[d],
                "pz": pzf if d == "f" else pzb,
            }

        def step(l, d, s, st):
            t = s if d == "f" else Tn - 1 - s
            hist = st["hist"]
            # fw: slot t+1 = h after t (init 0); bw: slot t = h after t (init Tn)
            sl_prev = t if d == "f" else t + 1
            sl_new = t + 1 if d == "f" else t

            if s % TPC == 0:
                # one chunk covers the next 4 timesteps of this stream,
                # timesteps along the free dim (base partition must be 0)
                q = t // TPC
                xwc = sp.tile([BL, TPC * G4], bf16, name=f"xwc{l}{d}{s}", tag=f"xwc{d}", bufs=2)
                nc.gpsimd.dma_start(
                    xwc[:].rearrange("b (t g) -> b t g", t=TPC),
                    xw[(l, d)][128 * q : 128 * (q + 1), :].rearrange(
                        "(t b) g -> b t g", b=BL
                    ),
                )
                st["xwc"] = xwc
            xwt = st["xwc"][:, (t % TPC) * G4 : (t % TPC + 1) * G4]

            zp = st["pz"].tile([BL, G4], f32, name=f"z{l}{d}{s}", tag=f"zp{d}")
            # xw-fold mms first in each group: they depend only on the (early)
            # xw load, so the PE runs them while waiting for the hist copy.
            # (i,f) slice first so sig_if (the longer ACT op) overlaps the
            # (o,j) slice's matmuls.
            for n in (0, 1):
                n0 = 512 * n
                nc.tensor.matmul(
                    zp[:, n0 : n0 + 512], id32[:], xwt[:, n0 : n0 + 512],
                    start=True, stop=False,
                )
            for n in (0, 1):
                n0 = 512 * n
                for k in range(NKH):
                    nc.tensor.matmul(
                        zp[:, n0 : n0 + 512],
                        hist[:, k, sl_prev, :],
                        whs[(l, d)][:, k * G4 + n0 : k * G4 + n0 + 512],
                        start=False, stop=(k == NKH - 1),
                        skip_group_check=True,
                    )

            gc = st["gc"]
            sio = sp.tile([BL, 3 * Hn], bf16, name=f"sio{l}{d}{s}", tag=f"sio{d}")
            nc.scalar.activation(sio[:, 0 : 2 * Hn], zp[:, 0 : 2 * Hn], AF.Sigmoid)
            nc.scalar.activation(gc[:, 0:Hn], zp[:, 3 * Hn : 4 * Hn], AF.Tanh)
            nc.scalar.activation(
                sio[:, 2 * Hn : 3 * Hn], zp[:, 2 * Hn : 3 * Hn], AF.Sigmoid
            )
            prods = sp.tile([BL, 2 * Hn], bf16, name=f"pr{l}{d}{s}", tag=f"pr{d}")
            nc.vector.tensor_tensor(prods[:], sio[:, 0 : 2 * Hn], gc[:], op=OP.mult)
            nc.vector.tensor_tensor(
                gc[:, Hn : 2 * Hn], prods[:, 0:Hn], prods[:, Hn : 2 * Hn], op=OP.add
            )
            th = sp.tile([BL, Hn], bf16, name=f"th{l}{d}{s}", tag=f"th{d}")
            nc.scalar.activation(th[:], gc[:, Hn : 2 * Hn], AF.Tanh)
            h = sp.tile([BL, Hn], bf16, name=f"h{l}{d}{s}", tag=f"h{d}")
            nc.vector.tensor_tensor(h[:], sio[:, 2 * Hn : 3 * Hn], th[:], op=OP.mult)

            tp = ps.tile([128, HT], bf16, name=f"tp{l}{d}{s}", tag="sm")
            for k in range(NKH):
                nc.tensor.matmul(
                    tp[:, BL * k : BL * (k + 1)],
                    h[:, 128 * k : 128 * (k + 1)],
                    id32[:],
                    is_transpose=True,
                )
            nc.vector.tensor_copy(
                hist[:, :, sl_new, :], tp[:].rearrange("p (k b) -> p k b", k=NKH)
            )


        # ---------------- logits chunk + unary accum
        usum = cp.tile([128, NCH], f32, name="usum", tag="usum")
        lgall = cp.tile([BL, Tn * Kn], f32, name="lgall", tag="lgall")

        def logits_chunk(c):
            lp = ps.tile([128, Kn], f32, name=f"lp{c}", tag="sm")
            nc.tensor.matmul(lp[:], ones1[:], dbs[:], start=True, stop=False)
            for k in range(NKX):
                fw = k < NKH
                src = hist1["f"] if fw else hist1["b"]
                s0 = TPC * c + (1 if fw else 0)
                nc.tensor.matmul(
                    lp[:],
                    src[:, k % NKH, s0 : s0 + TPC, :].rearrange("p t b -> p (t b)"),
                    dws[:, Kn * k : Kn * (k + 1)],
                    start=False, stop=(k == NKX - 1),
                )
            lgc = gp.tile([128, Kn], f32, name=f"lg{c}", tag="lgc", bufs=3)
            nc.vector.tensor_copy(lgc[:], lp[:])
            nc.sync.dma_start(lgd[128 * c : 128 * (c + 1), :], lgc[:])
            # gather this chunk into the CRF layout [b, (t, k)] (DRAM-routed:
            # SBUF->SBUF DMA dep tracking is unreliable)
            nc.sync.dma_start(
                lgall[:].rearrange("b (t k) -> b t k", k=Kn)[
                    :, TPC * c : TPC * (c + 1), :
                ],
                lgd[128 * c : 128 * (c + 1), :].rearrange("(t b) k -> b t k", b=BL),
            )
            oht = gp.tile([128, Kn], f32, name=f"oht{c}", tag="oht", bufs=3)
            nc.gpsimd.dma_start(oht[:], oh_d[128 * c : 128 * (c + 1), :])
            scr = gp.tile([128, Kn], f32, name=f"ohscr{c}", tag="ohscr", bufs=3)
            nc.vector.scalar_tensor_tensor(
                scr[:], lgc[:], 1.0, oht[:], OP.mult, OP.mult,
                accum_out=usum[:, c : c + 1],
            )

        # ---------------- CRF (scaled-exp domain)
        def crf():
            crfp = ctx.enter_context(tc.tile_pool(name="crf", bufs=2))
            expx = cp.tile([BL, Tn * Kn], f32, name="expx", tag="expx")
            nc.scalar.activation(expx[:], lgall[:], AF.Exp)
            ls = cp.tile([BL, 1], f32, name="ls", tag="ls")
            nc.vector.memset(ls[:], 0.0)
            a = crfp.tile([BL, Kn], f32, name="a0", tag="a")
            nc.vector.tensor_copy(a[:], expx[:, 0:Kn])
            for t in range(1, Tn):
                p81 = crfp.tile([BL, K2], f32, name=f"p81_{t}", tag="p81")
                nc.vector.tensor_tensor(
                    p81[:].rearrange("p (j i) -> p j i", i=Kn),
                    a[:].unsqueeze(1).broadcast_to([BL, Kn, Kn]),
                    e81[:].rearrange("p (j i) -> p j i", i=Kn),
                    op=OP.mult,
                )
                s9 = crfp.tile([BL, Kn], f32, name=f"s9_{t}", tag="s9")
                nc.vector.reduce_sum(
                    s9[:], p81[:].rearrange("p (j i) -> p j i", i=Kn), axis=AX.X
                )
                u = crfp.tile([BL, Kn], f32, name=f"u{t}", tag="u")
                nc.vector.tensor_tensor(u[:], s9[:], expx[:, Kn * t : Kn * (t + 1)], op=OP.mult)
                anew = crfp.tile([BL, Kn], f32, name=f"a{t}", tag="a")
                nc.vector.select(
                    anew[:], m9[:, Kn * t : Kn * (t + 1)], u[:], a[:]
                )
                a = anew
                if t % RESCALE == 0:
                    mx = crfp.tile([BL, 1], f32, name=f"mx{t}", tag="mx")
                    nc.vector.reduce_max(mx[:], a[:], axis=AX.X)
                    r = crfp.tile([BL, 1], f32, name=f"r{t}", tag="r")
                    nc.vector.reciprocal(r[:], mx[:])
                    ar = crfp.tile([BL, Kn], f32, name=f"ar{t}", tag="a")
                    nc.vector.tensor_scalar_mul(ar[:], a[:], r[:, 0:1])
                    a = ar
                    lmx = crfp.tile([BL, 1], f32, name=f"lmx{t}", tag="lmx")
                    nc.scalar.activation(lmx[:], mx[:], AF.Ln)
                    nc.vector.tensor_tensor(ls[:], ls[:], lmx[:], op=OP.add)

            # logZ = ls + ln(sum a)
            sa = crfp.tile([BL, 1], f32, name="sa", tag="sa")
            nc.vector.reduce_sum(sa[:], a[:], axis=AX.X)
            lsa = crfp.tile([BL, 1], f32, name="lsa", tag="lsa")
            nc.scalar.activation(lsa[:], sa[:], AF.Ln)
            lgz = crfp.tile([BL, 1], f32, name="lgz", tag="lgz")
            nc.vector.tensor_tensor(lgz[:], lsa[:], ls[:], op=OP.add)

            # gold path scores
            up = ps.tile([BL, NCH], f32, name="up", tag="sm")
            nc.tensor.matmul(up[:], sels[:], usum[:], start=True, stop=True)
            unary = crfp.tile([BL, 1], f32, name="unary", tag="unary")
            nc.vector.reduce_sum(unary[:], up[:], axis=AX.X)
            bp = ps.tile([BL, 1], f32, name="bp", tag="sm")
            nc.tensor.matmul(bp[:], c81s[:], tfs[:], start=True, stop=True)
            binry = crfp.tile([BL, 1], f32, name="binry", tag="binry")
            nc.scalar.copy(binry[:], bp[:])

            zb = crfp.tile([BL, 1], f32, name="zb", tag="zb")
            nc.vector.tensor_tensor(zb[:], lgz[:], unary[:], op=OP.subtract)
            nll = crfp.tile([BL, 1], f32, name="nll", tag="nll")
            nc.vector.tensor_tensor(nll[:], zb[:], binry[:], op=OP.subtract)
            pf = ps.tile([1, 1], f32, name="pf", tag="sm")
            nc.tensor.matmul(pf[:], nll[:], onesb[:], start=True, stop=True)
            osb = crfp.tile([1, 1], f32, name="osb", tag="osb")
            nc.scalar.copy(osb[:], pf[:])
            nc.sync.dma_start(out_d[:, :], osb[:])

        # ---------------- emission schedule
        PH = cfg.get("phase", 99)
        NSTEP = min(Tn, cfg.get("nsteps", Tn))

        def probe(src_ap):
            pt = cp.tile([1, 1], f32, name="probe", tag="probe")
            nc.sync.dma_start(pt[:], src_ap)
            nc.sync.dma_start(out_d[:, :], pt[:])

        PRE = 6
        # prologue: first layer-0 pre-GEMM chunks
        for ci in range(min(PRE, NCH)):
            pre_gemm(0, "f", ci, NKE)
            pre_gemm(0, "b", NCH - 1 - ci, NKE)

        nextci = [PRE]

        def l0_hook(s):
            if s % 2 == 1 and nextci[0] < NCH:
                ci = nextci[0]
                nextci[0] += 1
                pre_gemm(0, "f", ci, NKE)
                pre_gemm(0, "b", NCH - 1 - ci, NKE)
            if PH >= 3 and s >= 131 and (s - 131) % TPC == 0:
                j = (s - 131) // TPC
                for q in (NCH // 2 - 1 - j, NCH // 2 + j):
                    if 0 <= q < NCH:
                        pre_gemm(1, "f", q, NKX)
                        pre_gemm(1, "b", q, NKX)

        def l1_hook(s):
            if PH >= 4 and s >= 131 and (s - 131) % TPC == 0:
                j = (s - 131) // TPC
                for q in (NCH // 2 - 1 - j, NCH // 2 + j):
                    if 0 <= q < NCH:
                        logits_chunk(q)

        def run_layer(l, hook):
            sts = {d: make_stream(l, d) for d in ("f", "b")}
            for s in range(NSTEP):
                for d in ("f", "b"):
                    step(l, d, s, sts[d])
                hook(s)

        run_layer(0, l0_hook)
        hp0cm.__exit__(None, None, None)
        hp1 = ctx.enter_context(tc.tile_pool(name="hist1pool", bufs=1))
        for d in ("f", "b"):
            h1 = hp1.tile([128, (Tn + 1) * HT], bf16, name=f"hist1{d}", tag=f"hist1{d}")
            hist1[d] = h1[:].rearrange("p (k t b) -> p k t b", t=Tn + 1, k=NKH)
            nc.vector.memset(hist1[d][:, :, 0 if d == "f" else Tn], 0.0)
        if PH == 1:
            pr = cp.tile([1, 1], f32, name="prcv", tag="prcv")
            nc.vector.tensor_copy(pr[:], hist0["f"][0:1, 0, Tn, 0:1])
            nc.sync.dma_start(out_d[:, :], pr[:])
        if PH >= 3:
            run_layer(1, l1_hook)
        if PH == 4:
            probe(lgd[0:1, 0:1])
        if PH >= 5:
            crf()

    if split:
        _split_excess_waits(nc)
    return nc


# ---------------------------------------------------------------- host prep
def _prep_core(emb_c, lens_c, tgt_c, weights, cfg):
    Tn, BL, En, Hn, Kn = cfg["T"], cfg["BL"], cfg["E"], cfg["H"], cfg["K"]
    EP = -(-En // 128) * 128
    G4 = 4 * Hn
    HP = 2 * Hn
    NKE = EP // 128
    NKX = HP // 128
    ROWS = Tn * BL
    NCH = ROWS // 128
    K2 = Kn * Kn

    perm = np.concatenate(
        [np.arange(0, Hn), np.arange(2 * Hn, 3 * Hn),
         np.arange(3 * Hn, 4 * Hn), np.arange(Hn, 2 * Hn)]
    )

    def prep_wb(w, b):
        wp = np.ascontiguousarray(w[:, perm], np.float32)
        bp = b[perm].astype(np.float32).copy()
        bp[Hn : 2 * Hn] += 1.0
        return wp, bp

    def chunk_k(w, kpad):
        out = np.zeros((kpad, w.shape[1]), np.float32)
        out[: w.shape[0]] = w
        nk = kpad // 128
        return np.ascontiguousarray(
            out.reshape(nk, 128, w.shape[1]).transpose(1, 0, 2).reshape(128, -1)
        )

    d = {}
    et = emb_c.transpose(2, 1, 0).reshape(En, ROWS)
    embT = np.zeros((EP, ROWS), np.float32)
    embT[:En] = et
    embT[En] = 1.0  # ones row for layer-0 bias
    d["embT"] = embT.astype(np.float32)

    tt = np.arange(Tn)
    valid = tt[:, None] < lens_c[None, :]          # [T, BL]
    # mneg rows (t, b): chunk c covers t = 4c..4c+3; partition p = (t%4)*32+b
    mneg = np.where(valid, 0.0, NEG_BIG).astype(np.float32)  # [T, BL]
    d["mneg"] = np.ascontiguousarray(
        mneg.reshape(NCH, 128 // BL, BL).transpose(1, 2, 0).reshape(128, NCH)
    )
    d["id32"] = np.eye(BL, dtype=np.float32)
    d["ones1"] = np.ones((1, 128), np.float32)
    d["dwc"] = chunk_k(weights["dense_w"].astype(np.float32), HP)
    d["db"] = weights["dense_b"].reshape(1, Kn).astype(np.float32)

    trans = weights["trans"].astype(np.float32)
    # p81[p, j, i] = a_i * e81[(j, i)], reduce over i (AX.X = innermost):
    # s9[p, j] = sum_i a_i exp(trans[i, j])  ->  e81[(j, i)] = exp(trans)[i, j]
    d["e81"] = np.tile(np.exp(trans).T.reshape(1, K2), (BL, 1)).astype(np.float32)
    d["m9"] = np.repeat(
        valid.T.astype(np.uint8)[:, :, None], Kn, axis=2
    ).reshape(BL, Tn * Kn)  # [BL, (t, k)], uint8 (CopyPredicated needs int mask)

    ohm = np.zeros((ROWS, Kn), np.float32)
    r = tt[:, None] * BL + np.arange(BL)[None, :]
    ohm[r.ravel(), tgt_c.T.ravel()] = valid.astype(np.float32).ravel()
    d["oh"] = ohm
    c81 = np.zeros((K2, BL), np.float32)
    for b in range(BL):
        L = int(lens_c[b])
        for t in range(L - 1):
            c81[tgt_c[b, t] * Kn + tgt_c[b, t + 1], b] += 1.0
    d["c81t"] = c81
    d["sel"] = (np.arange(128)[:, None] % BL == np.arange(BL)[None, :]).astype(np.float32)
    d["transflat"] = trans.reshape(K2, 1)

    for l, (wfk, bfk, wbk, bbk, kin) in enumerate(
        (("w_fw0", "b_fw0", "w_bw0", "b_bw0", EP), ("w_fw1", "b_fw1", "w_bw1", "b_bw1", HP))
    ):
        for dd, (wk, bk) in (("f", (wfk, bfk)), ("b", (wbk, bbk))):
            w, b = prep_wb(weights[wk], weights[bk])
            nin = w.shape[0] - Hn
            wx_part = w[:nin].copy()
            wh_part = w[nin:]
            if l == 0:
                wx_pad = np.zeros((kin, G4), np.float32)
                wx_pad[:nin] = wx_part
                wx_pad[En] = b  # bias via ones row
                d[f"wx{l}{dd}"] = chunk_k(wx_pad, kin)
            else:
                d[f"wx{l}{dd}"] = chunk_k(wx_part, kin)
                d[f"b1{dd}"] = np.tile(b.reshape(1, G4), (128, 1))
            d[f"wh{l}{dd}"] = chunk_k(wh_part, Hn)
    return d


_BF16_INPUTS = {
    "embT", "id32", "ones1", "dwc", "db",
    "wx0f", "wx0b", "wx1f", "wx1b", "b1f", "b1b",
    "wh0f", "wh0b", "wh1f", "wh1b",
}


def _get_runner(cfg):
    key = ("runner", cfg["T"], cfg["BL"], cfg["n_cores"], cfg.get("phase", 99))
    if key in _CACHE:
        return _CACHE[key]
    nc = build_nc(cfg)
    from concourse import bass2jax

    n_cores = cfg["n_cores"]

    import jax
    import numpy as _np
    from jax.sharding import Mesh, PartitionSpec
    from jax.experimental.shard_map import shard_map

    bass2jax.install_neuronx_cc_hook()
    partition_name = nc.partition_id_tensor.name if nc.partition_id_tensor else None
    import concourse.mybir as mybir

    in_names, out_names, out_avals = [], [], []
    in_dtypes = {}
    for alloc in nc.m.functions[0].allocations:
        if not isinstance(alloc, mybir.MemoryLocationSet):
            continue
        name = alloc.memorylocations[0].name
        if alloc.kind == "ExternalInput":
            if name != partition_name:
                in_names.append(name)
                in_dtypes[name] = mybir.dt.np(alloc.dtype)
        elif alloc.kind == "ExternalOutput":
            out_names.append(name)
            out_avals.append(
                jax.core.ShapedArray(tuple(alloc.tensor_shape), mybir.dt.np(alloc.dtype))
            )
    n_params = len(in_names)
    all_names = in_names + out_names
    if partition_name is not None:
        all_names = all_names + [partition_name]
    donate = tuple(range(n_params, n_params + len(out_names)))

    def _body(*args):
        operands = list(args)
        if partition_name is not None:
            operands.append(bass2jax.partition_id_tensor())
        outs = bass2jax._bass_exec_p.bind(
            *operands,
            out_avals=tuple(out_avals),
            in_names=tuple(all_names),
            out_names=tuple(out_names),
            lowering_input_output_aliases=(),
            sim_require_finite=True,
            sim_require_nnan=True,
            nc=nc,
        )
        return tuple(outs)

    devices = jax.devices()[:n_cores]

    class Runner:
        pass

    r = Runner()
    r.in_names, r.out_names, r.out_avals, r.n_cores = in_names, out_names, out_avals, n_cores
    r.in_dtypes = in_dtypes
    if n_cores == 1:
        fn = jax.jit(_body, donate_argnums=donate, keep_unused=True)

        def pack(in_maps):
            return [np.asarray(in_maps[0][n], in_dtypes[n]) for n in in_names]

        def call(packed):
            zeros = [np.zeros(a.shape, a.dtype) for a in out_avals]
            outs = fn(*packed, *zeros)
            return [{n: np.asarray(outs[i]) for i, n in enumerate(out_names)}]
    else:
        from jax.sharding import NamedSharding

        mesh = Mesh(_np.asarray(devices), ("core",))
        fn = jax.jit(
            shard_map(
                _body,
                mesh=mesh,
                in_specs=(PartitionSpec("core"),) * (n_params + len(out_names)),
                out_specs=(PartitionSpec("core"),) * len(out_names),
                check_rep=False,
            ),
            donate_argnums=donate,
            keep_unused=True,
        )
        sh = NamedSharding(mesh, PartitionSpec("core"))

        def pack(in_maps):
            concat_in = [
                np.concatenate(
                    [np.asarray(m[n], in_dtypes[n]) for m in in_maps], axis=0
                )
                for n in in_names
            ]
            return [jax.device_put(a, sh) for a in concat_in]

        def call(packed):
            zeros = [
                np.zeros((n_cores * a.shape[0],) + tuple(a.shape[1:]), a.dtype)
                for a in out_avals
            ]
            outs = fn(*packed, *zeros)
            return [
                {
                    n: np.asarray(outs[i]).reshape((n_cores,) + tuple(out_avals[i].shape))[c]
                    for i, n in enumerate(out_names)
                }
                for c in range(n_cores)
            ]

    r.fn = fn
    r.pack = pack
    r.call = call

    def run(in_maps):
        return call(pack(in_maps))

    r.run = run
    _CACHE[key] = r
    return r


def make_in_maps(inputs, cfg):
    n_cores = cfg["n_cores"]
    BL = cfg["BL"]
    weights = {
        k: np.asarray(inputs[k], np.float32)
        for k in (
            "w_fw0", "b_fw0", "w_bw0", "b_bw0",
            "w_fw1", "b_fw1", "w_bw1", "b_bw1",
            "dense_w", "dense_b", "trans",
        )
    }
    emb = np.asarray(inputs["emb"], np.float32)
    lens = np.asarray(inputs["seq_lens"], np.int64)
    tgt = np.asarray(inputs["targets"], np.int64)
    in_maps = []
    for c in range(n_cores):
        sl = slice(c * BL, (c + 1) * BL)
        in_maps.append(_prep_core(emb[sl], lens[sl], tgt[sl], weights, cfg))
    return in_maps


def kernel(**inputs):
    cfg = dict(T=T, BL=B // N_CORES, E=E, H=H, K=K, n_cores=N_CORES)
    in_maps = make_in_maps(inputs, cfg)
    runner = _get_runner(cfg)
    res = runner.run(in_maps)
    total = sum(float(r["out"][0, 0]) for r in res)
    return np.asarray(np.float32(total / B))
